# revision 14
# baseline (speedup 1.0000x reference)
"""Trainium2 Bass kernel for nn_CrossAttention (B=4, C=512, H=W=64, CQK=64).

Math (per batch b):
    Q = Wq @ rgb + bq                      [CQK, HW]
    K = Wk @ chm + bk                      [CQK, XY]
    S[hw, xy] = sum_o Q[o, hw] K[o, xy]    (xy = x*64 + y)
    P = softmax over y only (last 64-group of xy)
    att[c, hw] = sum_xy P[hw, xy] V[c, xy],  V = Wv @ chm + bv
    out = rgb + gamma * att

Sharding: 8 cores = 4 batches x 2 halves of the hw (query) axis; each core
computes its 2048-query slice of the attention map and attended output
against the full 4096-key/value domain of its batch. No collectives needed.

The small 1x1-conv GEMMs (Q/K/V projections; see sharding hint) are folded
into host-side input prep, exactly:
  - qt = Wq @ rgb + bq, kf = Wk @ chm + bk (f32 GEMMs, cast bf16).
  - chm' = (gamma*Wv) @ chm, pre-transposed into attend-weight tiles, so the
    device attend GEMM produces gamma*att directly.
  - bv contributes exactly 64*gamma*bv[c] per output pixel (softmax rows sum
    to 1 per (hw, x), 64 x-groups), folded into the residual rgb operand.
The quadratic attention compute (S = Q^T K, softmax, attend) runs on device.

Device dataflow per core (bf16 matmuls, f32 PSUM accumulate), per 128-row
query tile ("htile"):
  - S on PE ([128, 1024] PSUM tiles), exp on ACT -> E bf16.
  - Z via DVE pairwise-tree sum over y; reciprocal; 1/Z broadcast-expanded on
    GPSIMD so the DVE normalize multiply gets packed operands (2x bf16 mode);
    P^T via DMA xbar transpose.
  - Attend chains (32 accumulating matmuls, N=128 columns) interleaved `lag`
    htiles behind the S/softmax pipeline so the PE never idles (idle gaps
    reset the p-state ramp). DVE adds the f32 rgb residual; per-htile stores.
DMA: everything with late semaphore waits (transposes, residual loads,
stores) on the SP HWDGE ring; the early dependency-free attend-weight loads
on the ACT ring, which otherwise stays exp-only so DMA waits can never block
the exp stream at the ACT sequencer. DRAM layouts are pre-arranged so bulk
loads are contiguous per partition.
"""

import numpy as np
import ml_dtypes

import concourse.bass as bass
import concourse.mybir as mybir
import concourse.tile as tile
from concourse import bacc
from concourse.bass_utils import run_bass_kernel_spmd

P = 128
B, C, H, W = 4, 512, 64, 64
HW = H * W                # 4096
CQK = C // 8              # 64
N_CORES = 8
HWC = HW // 2             # hw rows per core (2048)

F32 = mybir.dt.float32
BF16 = mybir.dt.bfloat16
ADD = mybir.AluOpType.add
MULT = mybir.AluOpType.mult
EXP = mybir.ActivationFunctionType.Exp

BF16NP = ml_dtypes.bfloat16


def build_program(hwc=HWC, xy=HW, c=C, cqk=CQK, n_cores=N_CORES, lag=7,
                  direct_head=2, direct_tail=2):
    """Build the per-core Bass program. Returns a compiled Bacc module."""
    ck = c // P               # channel chunks (4)
    nb = hwc // 512           # hw blocks (4)
    nh = hwc // P             # hw tiles (16)
    xt = xy // P              # xy tiles (32)
    y = 64                    # softmax group size
    xg = xy // y              # x values (64)

    nc = bacc.Bacc("TRN2", target_bir_lowering=False, debug=False,
                   num_devices=n_cores)
    ld = nc.sync
    st = nc.scalar

    qtd = nc.dram_tensor("qt", [cqk, hwc], BF16, kind="ExternalInput")
    kfd = nc.dram_tensor("kf", [cqk, xy], BF16, kind="ExternalInput")
    cvt = nc.dram_tensor("cvt", [P, ck * xt * P], BF16, kind="ExternalInput")
    rga = nc.dram_tensor("rga", [P, ck * hwc], F32, kind="ExternalInput")
    out = nc.dram_tensor("out", [P, ck * hwc], F32, kind="ExternalOutput")

    cvt_t = cvt.ap().rearrange("p (k t q) -> p k t q", k=ck, t=xt)
    rga_t = rga.ap().rearrange("p (k n) -> p k n", k=ck)
    out_t = out.ap().rearrange("p (k n) -> p k n", k=ck)

    with tile.TileContext(nc) as tc:
        with tc.tile_pool(name="pers", bufs=1) as pers:
            kf = pers.tile([cqk, xy], BF16)
            ld.dma_start(kf[:], kfd.ap())
            qt = pers.tile([cqk, hwc], BF16)
            ld.dma_start(qt[:], qtd.ap())
            # attend weights on the ACT ring so they can't delay kf/qt
            cvt_sb = pers.tile([P, ck, xt, P], BF16)
            for k in range(ck):
                st.dma_start(cvt_sb[:, k], cvt_t[:, k])

            with tc.tile_pool(name="pmain", bufs=5) as pmain, \
                 tc.tile_pool(name="zpool", bufs=1) as zpool, \
                 tc.tile_pool(name="rzpool", bufs=2) as rzpool, \
                 tc.tile_pool(name="rzbpool", bufs=2) as rzbpool, \
                 tc.tile_pool(name="ptpool", bufs=lag + 2) as ptpool, \
                 tc.tile_pool(name="rgf", bufs=2) as rgf, \
                 tc.tile_pool(name="opool", bufs=4) as opool, \
                 tc.tile_pool(name="psS", bufs=4, space="PSUM") as psS, \
                 tc.tile_pool(name="psA", bufs=4, space="PSUM") as psA, \
                 nc.allow_low_precision(reason="softmax weights in bf16"):

                def softmax_htile(h):
                    p_sb = pmain.tile([P, xy], BF16, tag="p")
                    for s in range(xy // 512):
                        s_ps = psS.tile([P, 512], F32, tag="sps")
                        nc.tensor.matmul(
                            s_ps[:],
                            qt[:, P * h:P * (h + 1)],
                            kf[:, 512 * s:512 * (s + 1)],
                            start=True, stop=True)
                        nc.scalar.activation(
                            p_sb[:, 512 * s:512 * (s + 1)], s_ps[:], EXP)
                    # Z = sum over y (pairwise tree, bf16), then 1/Z
                    v3 = p_sb[:].rearrange("p (x y) -> p x y", y=y)
                    tcur = v3
                    w = y
                    while w > 1:
                        w //= 2
                        tnext = zpool.tile([P, xg, w], BF16, tag=f"z{w}")
                        nc.vector.tensor_tensor(
                            tnext[:], tcur[:, :, 0:w], tcur[:, :, w:2 * w],
                            ADD)
                        tcur = tnext
                    rz = rzpool.tile([P, xg, 1], BF16, tag="rz")
                    nc.vector.reciprocal(rz[:], tcur[:])
                    if direct_head <= h < nh - direct_tail:
                        # expand 1/Z on GPSIMD so the DVE multiply gets packed
                        # operands (2x bf16 mode)
                        rzb = rzbpool.tile([P, xg, y], BF16, tag="rzb")
                        nc.gpsimd.tensor_copy(
                            rzb[:], rz[:].to_broadcast([P, xg, y]))
                        nc.vector.tensor_tensor(v3, v3, rzb[:], MULT)
                    else:
                        # head/tail htiles: skip the GPSIMD hop (its latency
                        # sits on the pipeline fill/drain critical path)
                        nc.vector.tensor_tensor(
                            v3, v3, rz[:].to_broadcast([P, xg, y]), MULT)
                    ptb = ptpool.tile([P, xt, P], BF16, tag="ptb")
                    nc.sync.dma_start(ptb[:], p_sb[:], transpose=True)
                    return ptb

                rg_blk = [None] * nb

                def attend_htile(g, ptb):
                    blk, ht = divmod(g, nb)
                    if ht == 0:
                        rg = rgf.tile([P, ck, 512], F32, tag="rg",
                                      name=f"rg{blk}")
                        ld.dma_start(rg[:],
                                     rga_t[:, :, 512 * blk:512 * (blk + 1)])
                        rg_blk[blk] = rg
                    rg = rg_blk[blk]
                    o_sb = opool.tile([P, ck, P], F32, tag="o")
                    cols = slice(P * ht, P * (ht + 1))
                    for ch in range(ck):
                        a_ps = psA.tile([P, P], F32, tag="aps")
                        for m in range(xt):
                            nc.tensor.matmul(
                                a_ps[:], cvt_sb[:, ch, m], ptb[:, m, :],
                                start=(m == 0), stop=(m == xt - 1))
                        nc.vector.tensor_tensor(o_sb[:, ch], a_ps[:],
                                                rg[:, ch, cols], ADD)
                    ld.dma_start(out_t[:, :, P * g:P * (g + 1)], o_sb[:])

                # attend before softmax within a round: the residual adds land
                # ahead of the next tree/mult in DVE program order, so attend
                # PSUM tiles recycle without head-of-line blocking
                ptbs = {}
                for h in range(nh):
                    if h >= lag:
                        attend_htile(h - lag, ptbs.pop(h - lag))
                    ptbs[h] = softmax_htile(h)
                for g in range(nh - lag, nh):
                    attend_htile(g, ptbs.pop(g))

    nc.compile()
    return nc


_NC_CACHE = {}


def _get_nc():
    if "nc" not in _NC_CACHE:
        _NC_CACHE["nc"] = build_program()
    return _NC_CACHE["nc"]


def make_in_maps(rgb_features, chm_features, Wq, bq, Wk, bk, Wv, bv, gamma):
    rgb_features = np.asarray(rgb_features, dtype=np.float32)
    chm_features = np.asarray(chm_features, dtype=np.float32)
    Wq = np.asarray(Wq, dtype=np.float32)
    Wk = np.asarray(Wk, dtype=np.float32)
    Wv = np.asarray(Wv, dtype=np.float32)
    bq = np.asarray(bq, dtype=np.float32).reshape(CQK, 1)
    bk = np.asarray(bk, dtype=np.float32).reshape(CQK, 1)
    bv = np.asarray(bv, dtype=np.float32)
    g = float(np.asarray(gamma).reshape(-1)[0])

    ck = C // P
    xt = HW // P
    # softmax rows sum to 1 per (hw, x); summing over the 64 x's makes the
    # bias term contribute exactly 64*gamma*bv[c] to every output pixel.
    rgb_adj = rgb_features + (64.0 * g * bv)[None, :, None, None]
    gwv = g * Wv

    in_maps = []
    per_batch = {}
    for core in range(N_CORES):
        b, half = divmod(core, 2)
        if b not in per_batch:
            chm_b = chm_features[b].reshape(C, HW)
            kf_d = (Wk @ chm_b + bk).astype(BF16NP)      # [CQK, XY]
            # chm' = (gamma Wv) @ chm, pre-transposed to the attend-weight
            # tile layout: cvt[p, k, t, q] = chm'[k*128+q, t*128+p]
            chmp = (gwv @ chm_b).astype(BF16NP)          # [C, XY]
            cvt_d = np.ascontiguousarray(
                chmp.reshape(ck, P, xt, P).transpose(3, 0, 2, 1)
                .reshape(P, ck * xt * P))
            per_batch[b] = (kf_d, cvt_d)
        kf_d, cvt_d = per_batch[b]

        sl = slice(half * HWC, (half + 1) * HWC)
        rgb_c = rgb_features[b].reshape(C, HW)[:, sl]
        qt_d = (Wq @ rgb_c + bq).astype(BF16NP)          # [CQK, HWC]
        rga_c = rgb_adj[b].reshape(C, HW)[:, sl]
        rga_d = np.ascontiguousarray(
            rga_c.reshape(ck, P, HWC).transpose(1, 0, 2).reshape(P, ck * HWC))
        in_maps.append({
            "qt": qt_d, "kf": kf_d, "cvt": cvt_d, "rga": rga_d,
        })
    return in_maps


def assemble(results):
    fused = np.empty((B, C, H, W), dtype=np.float32)
    fused2 = fused.reshape(B, C, HW)
    ck = C // P
    for core in range(N_CORES):
        b, half = divmod(core, 2)
        o = np.asarray(results[core]["out"], dtype=np.float32)
        o = o.reshape(P, ck, HWC).transpose(1, 0, 2).reshape(C, HWC)
        fused2[b, :, half * HWC:(half + 1) * HWC] = o
    return fused


def kernel(rgb_features, chm_features, Wq, bq, Wk, bk, Wv, bv, gamma):
    nc = _get_nc()
    in_maps = make_in_maps(rgb_features, chm_features, Wq, bq, Wk, bk, Wv, bv,
                           gamma)
    res = run_bass_kernel_spmd(nc, in_maps, core_ids=list(range(N_CORES)))
    return assemble(res.results)


# revision 15
# speedup vs baseline: 1.0199x; 1.0199x over previous
"""Trainium2 Bass kernel for nn_CrossAttention (B=4, C=512, H=W=64, CQK=64).

Math (per batch b):
    Q = Wq @ rgb + bq                      [CQK, HW]
    K = Wk @ chm + bk                      [CQK, XY]
    S[hw, xy] = sum_o Q[o, hw] K[o, xy]    (xy = x*64 + y)
    P = softmax over y only (last 64-group of xy)
    att[c, hw] = sum_xy P[hw, xy] V[c, xy],  V = Wv @ chm + bv
    out = rgb + gamma * att

Sharding: 8 cores = 4 batches x 2 halves of the hw (query) axis; each core
computes its 2048-query slice of the attention map and attended output
against the full 4096-key/value domain of its batch. No collectives needed.

The small 1x1-conv GEMMs (Q/K/V projections; see sharding hint) are folded
into host-side input prep, exactly:
  - qt = Wq @ rgb + bq, kf = Wk @ chm + bk (f32 GEMMs, cast bf16).
  - chm' = (gamma*Wv) @ chm, pre-transposed into attend-weight tiles, so the
    device attend GEMM produces gamma*att directly.
  - bv contributes exactly 64*gamma*bv[c] per output pixel (softmax rows sum
    to 1 per (hw, x), 64 x-groups), folded into the residual rgb operand.
The quadratic attention compute (S = Q^T K, softmax, attend) runs on device.

Device dataflow per core (bf16 matmuls, f32 PSUM accumulate), per 128-row
query tile ("htile"):
  - S on PE ([128, 1024] PSUM tiles), exp on ACT -> E bf16.
  - Z via DVE pairwise-tree sum over y; reciprocal; 1/Z broadcast-expanded on
    GPSIMD so the DVE normalize multiply gets packed operands (2x bf16 mode);
    P^T via DMA xbar transpose.
  - Attend chains (32 accumulating matmuls, N=128 columns) interleaved `lag`
    htiles behind the S/softmax pipeline so the PE never idles (idle gaps
    reset the p-state ramp). DVE adds the f32 rgb residual; per-htile stores.
DMA: everything with late semaphore waits (transposes, residual loads,
stores) on the SP HWDGE ring; the early dependency-free attend-weight loads
on the ACT ring, which otherwise stays exp-only so DMA waits can never block
the exp stream at the ACT sequencer. DRAM layouts are pre-arranged so bulk
loads are contiguous per partition.
"""

import numpy as np
import ml_dtypes

import concourse.bass as bass
import concourse.mybir as mybir
import concourse.tile as tile
from concourse import bacc
from concourse.bass_utils import run_bass_kernel_spmd

P = 128
B, C, H, W = 4, 512, 64, 64
HW = H * W                # 4096
CQK = C // 8              # 64
N_CORES = 8
HWC = HW // 2             # hw rows per core (2048)

F32 = mybir.dt.float32
BF16 = mybir.dt.bfloat16
ADD = mybir.AluOpType.add
MULT = mybir.AluOpType.mult
EXP = mybir.ActivationFunctionType.Exp

BF16NP = ml_dtypes.bfloat16


def build_program(hwc=HWC, xy=HW, c=C, cqk=CQK, n_cores=N_CORES, lag=6,
                  direct_head=2, direct_tail=2):
    """Build the per-core Bass program. Returns a compiled Bacc module."""
    ck = c // P               # channel chunks (4)
    nb = hwc // 512           # hw blocks (4)
    nh = hwc // P             # hw tiles (16)
    xt = xy // P              # xy tiles (32)
    y = 64                    # softmax group size
    xg = xy // y              # x values (64)

    nc = bacc.Bacc("TRN2", target_bir_lowering=False, debug=False,
                   num_devices=n_cores)
    ld = nc.sync
    st = nc.scalar

    qtd = nc.dram_tensor("qt", [cqk, hwc], BF16, kind="ExternalInput")
    kfd = nc.dram_tensor("kf", [cqk, xy], BF16, kind="ExternalInput")
    cvt = nc.dram_tensor("cvt", [P, ck * xt * P], BF16, kind="ExternalInput")
    rga = nc.dram_tensor("rga", [P, ck * hwc], F32, kind="ExternalInput")
    out = nc.dram_tensor("out", [P, ck * hwc], F32, kind="ExternalOutput")

    cvt_t = cvt.ap().rearrange("p (k t q) -> p k t q", k=ck, t=xt)
    rga_t = rga.ap().rearrange("p (k n) -> p k n", k=ck)
    out_t = out.ap().rearrange("p (k n) -> p k n", k=ck)

    with tile.TileContext(nc) as tc:
        with tc.tile_pool(name="pers", bufs=1) as pers:
            kf = pers.tile([cqk, xy], BF16)
            ld.dma_start(kf[:], kfd.ap())
            qt = pers.tile([cqk, hwc], BF16)
            ld.dma_start(qt[:], qtd.ap())
            # attend weights on the ACT ring so they can't delay kf/qt
            cvt_sb = pers.tile([P, ck, xt, P], BF16)
            for k in range(ck):
                st.dma_start(cvt_sb[:, k], cvt_t[:, k])

            with tc.tile_pool(name="pmain", bufs=6) as pmain, \
                 tc.tile_pool(name="zpool", bufs=1) as zpool, \
                 tc.tile_pool(name="rzpool", bufs=2) as rzpool, \
                 tc.tile_pool(name="rzbpool", bufs=2) as rzbpool, \
                 tc.tile_pool(name="ptpool", bufs=lag + 2) as ptpool, \
                 tc.tile_pool(name="rgf", bufs=2) as rgf, \
                 tc.tile_pool(name="opool", bufs=4) as opool, \
                 tc.tile_pool(name="psS", bufs=4, space="PSUM") as psS, \
                 tc.tile_pool(name="psA", bufs=4, space="PSUM") as psA, \
                 nc.allow_low_precision(reason="softmax weights in bf16"):

                def softmax_htile(h):
                    p_sb = pmain.tile([P, xy], BF16, tag="p")
                    for s in range(xy // 512):
                        s_ps = psS.tile([P, 512], F32, tag="sps")
                        nc.tensor.matmul(
                            s_ps[:],
                            qt[:, P * h:P * (h + 1)],
                            kf[:, 512 * s:512 * (s + 1)],
                            start=True, stop=True)
                        nc.scalar.activation(
                            p_sb[:, 512 * s:512 * (s + 1)], s_ps[:], EXP)
                    # Z = sum over y (pairwise tree, bf16), then 1/Z
                    v3 = p_sb[:].rearrange("p (x y) -> p x y", y=y)
                    tcur = v3
                    w = y
                    while w > 1:
                        w //= 2
                        tnext = zpool.tile([P, xg, w], BF16, tag=f"z{w}")
                        nc.vector.tensor_tensor(
                            tnext[:], tcur[:, :, 0:w], tcur[:, :, w:2 * w],
                            ADD)
                        tcur = tnext
                    rz = rzpool.tile([P, xg, 1], BF16, tag="rz")
                    nc.vector.reciprocal(rz[:], tcur[:])
                    if direct_head <= h < nh - direct_tail:
                        # expand 1/Z on GPSIMD so the DVE multiply gets packed
                        # operands (2x bf16 mode)
                        rzb = rzbpool.tile([P, xg, y], BF16, tag="rzb")
                        nc.gpsimd.tensor_copy(
                            rzb[:], rz[:].to_broadcast([P, xg, y]))
                        nc.vector.tensor_tensor(v3, v3, rzb[:], MULT)
                    else:
                        # head/tail htiles: skip the GPSIMD hop (its latency
                        # sits on the pipeline fill/drain critical path)
                        nc.vector.tensor_tensor(
                            v3, v3, rz[:].to_broadcast([P, xg, y]), MULT)
                    ptb = ptpool.tile([P, xt, P], BF16, tag="ptb")
                    nc.sync.dma_start(ptb[:], p_sb[:], transpose=True)
                    return ptb

                rg_blk = [None] * nb

                def attend_htile(g, ptb):
                    blk, ht = divmod(g, nb)
                    if ht == 0:
                        rg = rgf.tile([P, ck, 512], F32, tag="rg",
                                      name=f"rg{blk}")
                        ld.dma_start(rg[:],
                                     rga_t[:, :, 512 * blk:512 * (blk + 1)])
                        rg_blk[blk] = rg
                    rg = rg_blk[blk]
                    o_sb = opool.tile([P, ck, P], F32, tag="o")
                    cols = slice(P * ht, P * (ht + 1))
                    for ch in range(ck):
                        a_ps = psA.tile([P, P], F32, tag="aps")
                        for m in range(xt):
                            nc.tensor.matmul(
                                a_ps[:], cvt_sb[:, ch, m], ptb[:, m, :],
                                start=(m == 0), stop=(m == xt - 1))
                        nc.vector.tensor_tensor(o_sb[:, ch], a_ps[:],
                                                rg[:, ch, cols], ADD)
                    ld.dma_start(out_t[:, :, P * g:P * (g + 1)], o_sb[:])

                # attend before softmax within a round: the residual adds land
                # ahead of the next tree/mult in DVE program order, so attend
                # PSUM tiles recycle without head-of-line blocking
                ptbs = {}
                for h in range(nh):
                    if h >= lag:
                        attend_htile(h - lag, ptbs.pop(h - lag))
                    ptbs[h] = softmax_htile(h)
                for g in range(nh - lag, nh):
                    attend_htile(g, ptbs.pop(g))

    nc.compile()
    return nc


_NC_CACHE = {}


def _get_nc():
    if "nc" not in _NC_CACHE:
        _NC_CACHE["nc"] = build_program()
    return _NC_CACHE["nc"]


def make_in_maps(rgb_features, chm_features, Wq, bq, Wk, bk, Wv, bv, gamma):
    rgb_features = np.asarray(rgb_features, dtype=np.float32)
    chm_features = np.asarray(chm_features, dtype=np.float32)
    Wq = np.asarray(Wq, dtype=np.float32)
    Wk = np.asarray(Wk, dtype=np.float32)
    Wv = np.asarray(Wv, dtype=np.float32)
    bq = np.asarray(bq, dtype=np.float32).reshape(CQK, 1)
    bk = np.asarray(bk, dtype=np.float32).reshape(CQK, 1)
    bv = np.asarray(bv, dtype=np.float32)
    g = float(np.asarray(gamma).reshape(-1)[0])

    ck = C // P
    xt = HW // P
    # softmax rows sum to 1 per (hw, x); summing over the 64 x's makes the
    # bias term contribute exactly 64*gamma*bv[c] to every output pixel.
    rgb_adj = rgb_features + (64.0 * g * bv)[None, :, None, None]
    gwv = g * Wv

    in_maps = []
    per_batch = {}
    for core in range(N_CORES):
        b, half = divmod(core, 2)
        if b not in per_batch:
            chm_b = chm_features[b].reshape(C, HW)
            kf_d = (Wk @ chm_b + bk).astype(BF16NP)      # [CQK, XY]
            # chm' = (gamma Wv) @ chm, pre-transposed to the attend-weight
            # tile layout: cvt[p, k, t, q] = chm'[k*128+q, t*128+p]
            chmp = (gwv @ chm_b).astype(BF16NP)          # [C, XY]
            cvt_d = np.ascontiguousarray(
                chmp.reshape(ck, P, xt, P).transpose(3, 0, 2, 1)
                .reshape(P, ck * xt * P))
            per_batch[b] = (kf_d, cvt_d)
        kf_d, cvt_d = per_batch[b]

        sl = slice(half * HWC, (half + 1) * HWC)
        rgb_c = rgb_features[b].reshape(C, HW)[:, sl]
        qt_d = (Wq @ rgb_c + bq).astype(BF16NP)          # [CQK, HWC]
        rga_c = rgb_adj[b].reshape(C, HW)[:, sl]
        rga_d = np.ascontiguousarray(
            rga_c.reshape(ck, P, HWC).transpose(1, 0, 2).reshape(P, ck * HWC))
        in_maps.append({
            "qt": qt_d, "kf": kf_d, "cvt": cvt_d, "rga": rga_d,
        })
    return in_maps


def assemble(results):
    fused = np.empty((B, C, H, W), dtype=np.float32)
    fused2 = fused.reshape(B, C, HW)
    ck = C // P
    for core in range(N_CORES):
        b, half = divmod(core, 2)
        o = np.asarray(results[core]["out"], dtype=np.float32)
        o = o.reshape(P, ck, HWC).transpose(1, 0, 2).reshape(C, HWC)
        fused2[b, :, half * HWC:(half + 1) * HWC] = o
    return fused


def kernel(rgb_features, chm_features, Wq, bq, Wk, bk, Wv, bv, gamma):
    nc = _get_nc()
    in_maps = make_in_maps(rgb_features, chm_features, Wq, bq, Wk, bk, Wv, bv,
                           gamma)
    res = run_bass_kernel_spmd(nc, in_maps, core_ids=list(range(N_CORES)))
    return assemble(res.results)


# revision 17
# speedup vs baseline: 1.1307x; 1.1087x over previous
"""Trainium2 Bass kernel for nn_CrossAttention (B=4, C=512, H=W=64, CQK=64).

Math (per batch b):
    Q = Wq @ rgb + bq                      [CQK, HW]
    K = Wk @ chm + bk                      [CQK, XY]
    S[hw, xy] = sum_o Q[o, hw] K[o, xy]    (xy = x*64 + y)
    P = softmax over y only (last 64-group of xy)
    att[c, hw] = sum_xy P[hw, xy] V[c, xy],  V = Wv @ chm + bv
    out = rgb + gamma * att

Sharding: 8 cores = 4 batches x 2 halves of the hw (query) axis; each core
computes its 2048-query slice of the attention map and attended output
against the full 4096-key/value domain of its batch. No collectives needed.

The small 1x1-conv GEMMs (Q/K/V projections; see sharding hint) are folded
into host-side input prep, exactly:
  - qt = Wq @ rgb + bq, kf = Wk @ chm + bk (f32 GEMMs, cast bf16).
  - chm' = (gamma*Wv) @ chm, pre-transposed into attend-weight tiles, so the
    device attend GEMM produces gamma*att directly.
  - bv contributes exactly 64*gamma*bv[c] per output pixel (softmax rows sum
    to 1 per (hw, x), 64 x-groups), folded into the residual rgb operand.
The quadratic attention compute (S = Q^T K, softmax, attend) runs on device.

Device dataflow per core (bf16 matmuls, f32 PSUM accumulate), per 128-row
query tile ("htile"):
  - S on PE ([128, 1024] PSUM tiles), exp on ACT -> E bf16.
  - Z via DVE pairwise-tree sum over y; reciprocal; 1/Z broadcast-expanded on
    GPSIMD so the DVE normalize multiply gets packed operands (2x bf16 mode);
    P^T via DMA xbar transpose.
  - Attend chains (32 accumulating matmuls, N=128 columns) interleaved `lag`
    htiles behind the S/softmax pipeline so the PE never idles (idle gaps
    reset the p-state ramp). DVE adds the f32 rgb residual; per-htile stores.
DMA: everything with late semaphore waits (transposes, residual loads,
stores) on the SP HWDGE ring; the early dependency-free attend-weight loads
on the ACT ring, which otherwise stays exp-only so DMA waits can never block
the exp stream at the ACT sequencer. DRAM layouts are pre-arranged so bulk
loads are contiguous per partition.
"""

import numpy as np
import ml_dtypes

import concourse.bass as bass
import concourse.mybir as mybir
import concourse.tile as tile
from concourse import bacc
from concourse.bass_utils import run_bass_kernel_spmd

P = 128
B, C, H, W = 4, 512, 64, 64
HW = H * W                # 4096
CQK = C // 8              # 64
N_CORES = 8
HWC = HW // 2             # hw rows per core (2048)

F32 = mybir.dt.float32
BF16 = mybir.dt.bfloat16
ADD = mybir.AluOpType.add
MULT = mybir.AluOpType.mult
EXP = mybir.ActivationFunctionType.Exp

BF16NP = ml_dtypes.bfloat16


def build_program(hwc=HWC, xy=HW, c=C, cqk=CQK, n_cores=N_CORES, lag=5,
                  direct_head=2, direct_tail=2):
    """Build the per-core Bass program. Returns a compiled Bacc module."""
    ck = c // P               # channel chunks (4)
    nb = hwc // 512           # hw blocks (4)
    nh = hwc // P             # hw tiles (16)
    xt = xy // P              # xy tiles (32)
    y = 64                    # softmax group size
    xg = xy // y              # x values (64)

    nc = bacc.Bacc("TRN2", target_bir_lowering=False, debug=False,
                   num_devices=n_cores)
    ld = nc.sync
    st = nc.scalar

    qtd = nc.dram_tensor("qt", [cqk, hwc], BF16, kind="ExternalInput")
    kfd = nc.dram_tensor("kf", [cqk, xy], BF16, kind="ExternalInput")
    cvt = nc.dram_tensor("cvt", [P, ck * xt * P], BF16, kind="ExternalInput")
    rga = nc.dram_tensor("rga", [P, ck * hwc], F32, kind="ExternalInput")
    out = nc.dram_tensor("out", [P, ck * hwc], F32, kind="ExternalOutput")

    cvt_t = cvt.ap().rearrange("p (k t q) -> p k t q", k=ck, t=xt)
    rga_t = rga.ap().rearrange("p (k n) -> p k n", k=ck)
    out_t = out.ap().rearrange("p (k n) -> p k n", k=ck)

    with tile.TileContext(nc) as tc:
        with tc.tile_pool(name="pers", bufs=1) as pers:
            kf = pers.tile([cqk, xy], BF16)
            ld.dma_start(kf[:], kfd.ap())
            qt = pers.tile([cqk, hwc], BF16)
            ld.dma_start(qt[:], qtd.ap())
            # attend weights on the ACT ring so they can't delay kf/qt
            cvt_sb = pers.tile([P, ck, xt, P], BF16)
            for k in range(ck):
                st.dma_start(cvt_sb[:, k], cvt_t[:, k])

            with tc.tile_pool(name="pmain", bufs=6) as pmain, \
                 tc.tile_pool(name="zpool", bufs=1) as zpool, \
                 tc.tile_pool(name="rzpool", bufs=2) as rzpool, \
                 tc.tile_pool(name="ptpool", bufs=lag + 3) as ptpool, \
                 tc.tile_pool(name="rgf", bufs=2) as rgf, \
                 tc.tile_pool(name="opool", bufs=3) as opool, \
                 tc.tile_pool(name="psS", bufs=4, space="PSUM") as psS, \
                 tc.tile_pool(name="psA", bufs=4, space="PSUM") as psA, \
                 nc.allow_low_precision(reason="softmax weights in bf16"):

                def softmax_htile(h):
                    p_sb = pmain.tile([P, xy], BF16, tag="p")
                    for s in range(xy // 512):
                        s_ps = psS.tile([P, 512], F32, tag="sps")
                        nc.tensor.matmul(
                            s_ps[:],
                            qt[:, P * h:P * (h + 1)],
                            kf[:, 512 * s:512 * (s + 1)],
                            start=True, stop=True)
                        nc.scalar.activation(
                            p_sb[:, 512 * s:512 * (s + 1)], s_ps[:], EXP)
                    # Z = sum over y (pairwise tree, bf16), then 1/Z
                    v3 = p_sb[:].rearrange("p (x y) -> p x y", y=y)
                    tcur = v3
                    w = y
                    while w > 1:
                        w //= 2
                        tnext = zpool.tile([P, xg, w], BF16, tag=f"z{w}")
                        nc.vector.tensor_tensor(
                            tnext[:], tcur[:, :, 0:w], tcur[:, :, w:2 * w],
                            ADD)
                        tcur = tnext
                    rz = rzpool.tile([P, xg, 1], BF16, tag="rz")
                    nc.vector.reciprocal(rz[:], tcur[:])
                    # normalize directly on DVE: the broadcast operand
                    # forfeits the 2x mode, but keeping the multiply off
                    # GPSIMD shortens the chain to the transpose, which paces
                    # the attend lag
                    nc.vector.tensor_tensor(
                        v3, v3, rz[:].to_broadcast([P, xg, y]), MULT)
                    ptb = ptpool.tile([P, xt, P], BF16, tag="ptb")
                    nc.sync.dma_start(ptb[:], p_sb[:], transpose=True)
                    return ptb

                rg_blk = [None] * nb

                def attend_htile(g, ptb):
                    blk, ht = divmod(g, nb)
                    if ht == 0:
                        rg = rgf.tile([P, ck, 512], F32, tag="rg",
                                      name=f"rg{blk}")
                        ld.dma_start(rg[:],
                                     rga_t[:, :, 512 * blk:512 * (blk + 1)])
                        rg_blk[blk] = rg
                    rg = rg_blk[blk]
                    o_sb = opool.tile([P, ck, P], F32, tag="o")
                    cols = slice(P * ht, P * (ht + 1))
                    for ch in range(ck):
                        a_ps = psA.tile([P, P], F32, tag="aps")
                        for m in range(xt):
                            nc.tensor.matmul(
                                a_ps[:], cvt_sb[:, ch, m], ptb[:, m, :],
                                start=(m == 0), stop=(m == xt - 1))
                        nc.vector.tensor_tensor(o_sb[:, ch], a_ps[:],
                                                rg[:, ch, cols], ADD)
                    ld.dma_start(out_t[:, :, P * g:P * (g + 1)], o_sb[:])

                # attend before softmax within a round: the residual adds land
                # ahead of the next tree/mult in DVE program order, so attend
                # PSUM tiles recycle without head-of-line blocking
                ptbs = {}
                for h in range(nh):
                    if h >= lag:
                        attend_htile(h - lag, ptbs.pop(h - lag))
                    ptbs[h] = softmax_htile(h)
                for g in range(nh - lag, nh):
                    attend_htile(g, ptbs.pop(g))

    nc.compile()
    return nc


_NC_CACHE = {}


def _get_nc():
    if "nc" not in _NC_CACHE:
        _NC_CACHE["nc"] = build_program()
    return _NC_CACHE["nc"]


def make_in_maps(rgb_features, chm_features, Wq, bq, Wk, bk, Wv, bv, gamma):
    rgb_features = np.asarray(rgb_features, dtype=np.float32)
    chm_features = np.asarray(chm_features, dtype=np.float32)
    Wq = np.asarray(Wq, dtype=np.float32)
    Wk = np.asarray(Wk, dtype=np.float32)
    Wv = np.asarray(Wv, dtype=np.float32)
    bq = np.asarray(bq, dtype=np.float32).reshape(CQK, 1)
    bk = np.asarray(bk, dtype=np.float32).reshape(CQK, 1)
    bv = np.asarray(bv, dtype=np.float32)
    g = float(np.asarray(gamma).reshape(-1)[0])

    ck = C // P
    xt = HW // P
    # softmax rows sum to 1 per (hw, x); summing over the 64 x's makes the
    # bias term contribute exactly 64*gamma*bv[c] to every output pixel.
    rgb_adj = rgb_features + (64.0 * g * bv)[None, :, None, None]
    gwv = g * Wv

    in_maps = []
    per_batch = {}
    for core in range(N_CORES):
        b, half = divmod(core, 2)
        if b not in per_batch:
            chm_b = chm_features[b].reshape(C, HW)
            kf_d = (Wk @ chm_b + bk).astype(BF16NP)      # [CQK, XY]
            # chm' = (gamma Wv) @ chm, pre-transposed to the attend-weight
            # tile layout: cvt[p, k, t, q] = chm'[k*128+q, t*128+p]
            chmp = (gwv @ chm_b).astype(BF16NP)          # [C, XY]
            cvt_d = np.ascontiguousarray(
                chmp.reshape(ck, P, xt, P).transpose(3, 0, 2, 1)
                .reshape(P, ck * xt * P))
            per_batch[b] = (kf_d, cvt_d)
        kf_d, cvt_d = per_batch[b]

        sl = slice(half * HWC, (half + 1) * HWC)
        rgb_c = rgb_features[b].reshape(C, HW)[:, sl]
        qt_d = (Wq @ rgb_c + bq).astype(BF16NP)          # [CQK, HWC]
        rga_c = rgb_adj[b].reshape(C, HW)[:, sl]
        rga_d = np.ascontiguousarray(
            rga_c.reshape(ck, P, HWC).transpose(1, 0, 2).reshape(P, ck * HWC))
        in_maps.append({
            "qt": qt_d, "kf": kf_d, "cvt": cvt_d, "rga": rga_d,
        })
    return in_maps


def assemble(results):
    fused = np.empty((B, C, H, W), dtype=np.float32)
    fused2 = fused.reshape(B, C, HW)
    ck = C // P
    for core in range(N_CORES):
        b, half = divmod(core, 2)
        o = np.asarray(results[core]["out"], dtype=np.float32)
        o = o.reshape(P, ck, HWC).transpose(1, 0, 2).reshape(C, HWC)
        fused2[b, :, half * HWC:(half + 1) * HWC] = o
    return fused


def kernel(rgb_features, chm_features, Wq, bq, Wk, bk, Wv, bv, gamma):
    nc = _get_nc()
    in_maps = make_in_maps(rgb_features, chm_features, Wq, bq, Wk, bk, Wv, bv,
                           gamma)
    res = run_bass_kernel_spmd(nc, in_maps, core_ids=list(range(N_CORES)))
    return assemble(res.results)


# revision 18
# speedup vs baseline: 1.1445x; 1.0122x over previous
"""Trainium2 Bass kernel for nn_CrossAttention (B=4, C=512, H=W=64, CQK=64).

Math (per batch b):
    Q = Wq @ rgb + bq                      [CQK, HW]
    K = Wk @ chm + bk                      [CQK, XY]
    S[hw, xy] = sum_o Q[o, hw] K[o, xy]    (xy = x*64 + y)
    P = softmax over y only (last 64-group of xy)
    att[c, hw] = sum_xy P[hw, xy] V[c, xy],  V = Wv @ chm + bv
    out = rgb + gamma * att

Sharding: 8 cores = 4 batches x 2 halves of the hw (query) axis; each core
computes its 2048-query slice of the attention map and attended output
against the full 4096-key/value domain of its batch. No collectives needed.

The small 1x1-conv GEMMs (Q/K/V projections; see sharding hint) are folded
into host-side input prep, exactly:
  - qt = Wq @ rgb + bq, kf = Wk @ chm + bk (f32 GEMMs, cast bf16).
  - chm' = (gamma*Wv) @ chm, pre-transposed into attend-weight tiles, so the
    device attend GEMM produces gamma*att directly.
  - bv contributes exactly 64*gamma*bv[c] per output pixel (softmax rows sum
    to 1 per (hw, x), 64 x-groups), folded into the residual rgb operand.
The quadratic attention compute (S = Q^T K, softmax, attend) runs on device.

Device dataflow per core (bf16 matmuls, f32 PSUM accumulate), per 128-row
query tile ("htile"):
  - S on PE ([128, 1024] PSUM tiles), exp on ACT -> E bf16.
  - Z via DVE pairwise-tree sum over y; reciprocal; 1/Z broadcast-expanded on
    GPSIMD so the DVE normalize multiply gets packed operands (2x bf16 mode);
    P^T via DMA xbar transpose.
  - Attend chains (32 accumulating matmuls, N=128 columns) interleaved `lag`
    htiles behind the S/softmax pipeline so the PE never idles (idle gaps
    reset the p-state ramp). DVE adds the f32 rgb residual; per-htile stores.
DMA: everything with late semaphore waits (transposes, residual loads,
stores) on the SP HWDGE ring; the early dependency-free attend-weight loads
on the ACT ring, which otherwise stays exp-only so DMA waits can never block
the exp stream at the ACT sequencer. DRAM layouts are pre-arranged so bulk
loads are contiguous per partition.
"""

import numpy as np
import ml_dtypes

import concourse.bass as bass
import concourse.mybir as mybir
import concourse.tile as tile
from concourse import bacc
from concourse.bass_utils import run_bass_kernel_spmd

P = 128
B, C, H, W = 4, 512, 64, 64
HW = H * W                # 4096
CQK = C // 8              # 64
N_CORES = 8
HWC = HW // 2             # hw rows per core (2048)

F32 = mybir.dt.float32
BF16 = mybir.dt.bfloat16
ADD = mybir.AluOpType.add
MULT = mybir.AluOpType.mult
EXP = mybir.ActivationFunctionType.Exp

BF16NP = ml_dtypes.bfloat16


def build_program(hwc=HWC, xy=HW, c=C, cqk=CQK, n_cores=N_CORES, lag=4,
                  direct_head=2, direct_tail=2):
    """Build the per-core Bass program. Returns a compiled Bacc module."""
    ck = c // P               # channel chunks (4)
    nb = hwc // 512           # hw blocks (4)
    nh = hwc // P             # hw tiles (16)
    xt = xy // P              # xy tiles (32)
    y = 64                    # softmax group size
    xg = xy // y              # x values (64)

    nc = bacc.Bacc("TRN2", target_bir_lowering=False, debug=False,
                   num_devices=n_cores)
    ld = nc.sync
    st = nc.scalar

    qtd = nc.dram_tensor("qt", [cqk, hwc], BF16, kind="ExternalInput")
    kfd = nc.dram_tensor("kf", [cqk, xy], BF16, kind="ExternalInput")
    cvt = nc.dram_tensor("cvt", [P, ck * xt * P], BF16, kind="ExternalInput")
    rga = nc.dram_tensor("rga", [P, ck * hwc], F32, kind="ExternalInput")
    out = nc.dram_tensor("out", [P, ck * hwc], F32, kind="ExternalOutput")

    cvt_t = cvt.ap().rearrange("p (k t q) -> p k t q", k=ck, t=xt)
    rga_t = rga.ap().rearrange("p (k n) -> p k n", k=ck)
    out_t = out.ap().rearrange("p (k n) -> p k n", k=ck)

    with tile.TileContext(nc) as tc:
        with tc.tile_pool(name="pers", bufs=1) as pers:
            # one FIFO ring for everything: qt/kf first (S(0) blocks on
            # them), then the attend-weight chunks, so nothing can race
            # ahead of the critical first loads at the DMA engines
            qt = pers.tile([cqk, hwc], BF16)
            ld.dma_start(qt[:], qtd.ap())
            kf = pers.tile([cqk, xy], BF16)
            ld.dma_start(kf[:], kfd.ap())
            cvt_sb = pers.tile([P, ck, xt, P], BF16)
            for k in range(ck):
                ld.dma_start(cvt_sb[:, k], cvt_t[:, k])

            with tc.tile_pool(name="pmain", bufs=6) as pmain, \
                 tc.tile_pool(name="zpool", bufs=1) as zpool, \
                 tc.tile_pool(name="rzpool", bufs=2) as rzpool, \
                 tc.tile_pool(name="ptpool", bufs=lag + 3) as ptpool, \
                 tc.tile_pool(name="rgf", bufs=2) as rgf, \
                 tc.tile_pool(name="opool", bufs=3) as opool, \
                 tc.tile_pool(name="psS", bufs=5, space="PSUM") as psS, \
                 tc.tile_pool(name="psA", bufs=3, space="PSUM") as psA, \
                 nc.allow_low_precision(reason="softmax weights in bf16"):

                def softmax_htile(h):
                    p_sb = pmain.tile([P, xy], BF16, tag="p")
                    for s in range(xy // 512):
                        s_ps = psS.tile([P, 512], F32, tag="sps")
                        nc.tensor.matmul(
                            s_ps[:],
                            qt[:, P * h:P * (h + 1)],
                            kf[:, 512 * s:512 * (s + 1)],
                            start=True, stop=True)
                        nc.scalar.activation(
                            p_sb[:, 512 * s:512 * (s + 1)], s_ps[:], EXP)
                    # Z = sum over y (pairwise tree, bf16), then 1/Z
                    v3 = p_sb[:].rearrange("p (x y) -> p x y", y=y)
                    tcur = v3
                    w = y
                    while w > 1:
                        w //= 2
                        tnext = zpool.tile([P, xg, w], BF16, tag=f"z{w}")
                        nc.vector.tensor_tensor(
                            tnext[:], tcur[:, :, 0:w], tcur[:, :, w:2 * w],
                            ADD)
                        tcur = tnext
                    rz = rzpool.tile([P, xg, 1], BF16, tag="rz")
                    nc.vector.reciprocal(rz[:], tcur[:])
                    # normalize directly on DVE: the broadcast operand
                    # forfeits the 2x mode, but keeping the multiply off
                    # GPSIMD shortens the chain to the transpose, which paces
                    # the attend lag
                    nc.vector.tensor_tensor(
                        v3, v3, rz[:].to_broadcast([P, xg, y]), MULT)
                    ptb = ptpool.tile([P, xt, P], BF16, tag="ptb")
                    nc.sync.dma_start(ptb[:], p_sb[:], transpose=True)
                    return ptb

                rg_blk = [None] * nb

                def attend_htile(g, ptb):
                    blk, ht = divmod(g, nb)
                    if ht == 0:
                        rg = rgf.tile([P, ck, 512], F32, tag="rg",
                                      name=f"rg{blk}")
                        ld.dma_start(rg[:],
                                     rga_t[:, :, 512 * blk:512 * (blk + 1)])
                        rg_blk[blk] = rg
                    rg = rg_blk[blk]
                    o_sb = opool.tile([P, ck, P], F32, tag="o")
                    cols = slice(P * ht, P * (ht + 1))
                    for ch in range(ck):
                        a_ps = psA.tile([P, P], F32, tag="aps")
                        for m in range(xt):
                            nc.tensor.matmul(
                                a_ps[:], cvt_sb[:, ch, m], ptb[:, m, :],
                                start=(m == 0), stop=(m == xt - 1))
                        nc.vector.tensor_tensor(o_sb[:, ch], a_ps[:],
                                                rg[:, ch, cols], ADD)
                    ld.dma_start(out_t[:, :, P * g:P * (g + 1)], o_sb[:])

                # attend before softmax within a round: the residual adds land
                # ahead of the next tree/mult in DVE program order, so attend
                # PSUM tiles recycle without head-of-line blocking
                ptbs = {}
                for h in range(nh):
                    if h >= lag:
                        attend_htile(h - lag, ptbs.pop(h - lag))
                    ptbs[h] = softmax_htile(h)
                for g in range(nh - lag, nh):
                    attend_htile(g, ptbs.pop(g))

    nc.compile()
    return nc


_NC_CACHE = {}


def _get_nc():
    if "nc" not in _NC_CACHE:
        _NC_CACHE["nc"] = build_program()
    return _NC_CACHE["nc"]


def make_in_maps(rgb_features, chm_features, Wq, bq, Wk, bk, Wv, bv, gamma):
    rgb_features = np.asarray(rgb_features, dtype=np.float32)
    chm_features = np.asarray(chm_features, dtype=np.float32)
    Wq = np.asarray(Wq, dtype=np.float32)
    Wk = np.asarray(Wk, dtype=np.float32)
    Wv = np.asarray(Wv, dtype=np.float32)
    bq = np.asarray(bq, dtype=np.float32).reshape(CQK, 1)
    bk = np.asarray(bk, dtype=np.float32).reshape(CQK, 1)
    bv = np.asarray(bv, dtype=np.float32)
    g = float(np.asarray(gamma).reshape(-1)[0])

    ck = C // P
    xt = HW // P
    # softmax rows sum to 1 per (hw, x); summing over the 64 x's makes the
    # bias term contribute exactly 64*gamma*bv[c] to every output pixel.
    rgb_adj = rgb_features + (64.0 * g * bv)[None, :, None, None]
    gwv = g * Wv

    in_maps = []
    per_batch = {}
    for core in range(N_CORES):
        b, half = divmod(core, 2)
        if b not in per_batch:
            chm_b = chm_features[b].reshape(C, HW)
            kf_d = (Wk @ chm_b + bk).astype(BF16NP)      # [CQK, XY]
            # chm' = (gamma Wv) @ chm, pre-transposed to the attend-weight
            # tile layout: cvt[p, k, t, q] = chm'[k*128+q, t*128+p]
            chmp = (gwv @ chm_b).astype(BF16NP)          # [C, XY]
            cvt_d = np.ascontiguousarray(
                chmp.reshape(ck, P, xt, P).transpose(3, 0, 2, 1)
                .reshape(P, ck * xt * P))
            per_batch[b] = (kf_d, cvt_d)
        kf_d, cvt_d = per_batch[b]

        sl = slice(half * HWC, (half + 1) * HWC)
        rgb_c = rgb_features[b].reshape(C, HW)[:, sl]
        qt_d = (Wq @ rgb_c + bq).astype(BF16NP)          # [CQK, HWC]
        rga_c = rgb_adj[b].reshape(C, HW)[:, sl]
        rga_d = np.ascontiguousarray(
            rga_c.reshape(ck, P, HWC).transpose(1, 0, 2).reshape(P, ck * HWC))
        in_maps.append({
            "qt": qt_d, "kf": kf_d, "cvt": cvt_d, "rga": rga_d,
        })
    return in_maps


def assemble(results):
    fused = np.empty((B, C, H, W), dtype=np.float32)
    fused2 = fused.reshape(B, C, HW)
    ck = C // P
    for core in range(N_CORES):
        b, half = divmod(core, 2)
        o = np.asarray(results[core]["out"], dtype=np.float32)
        o = o.reshape(P, ck, HWC).transpose(1, 0, 2).reshape(C, HWC)
        fused2[b, :, half * HWC:(half + 1) * HWC] = o
    return fused


def kernel(rgb_features, chm_features, Wq, bq, Wk, bk, Wv, bv, gamma):
    nc = _get_nc()
    in_maps = make_in_maps(rgb_features, chm_features, Wq, bq, Wk, bk, Wv, bv,
                           gamma)
    res = run_bass_kernel_spmd(nc, in_maps, core_ids=list(range(N_CORES)))
    return assemble(res.results)
